# revision 21
# baseline (speedup 1.0000x reference)
"""Trainium2 Bass kernel for LocalSparseAttention (anti-local windowed attention).

Reference computation (B=2, L=2048, D=512, H=8, hd=64):
    qkv = x @ in_proj_w.T + in_proj_b ; q,k,v = split(qkv)
    q *= 1/sqrt(hd)
    scores = q @ k.T  per head, with positions j in [i-w/2, i+w/2) BANNED (-inf)
    attn = softmax(scores); ctx = attn @ v
    out = LayerNorm(x + ctx @ out_proj_w.T + out_proj_b) * gamma + beta

Sharding: 8 cores = 2 batches x 4 query-shards of 512 rows. Each core
computes k/v for all 2048 keys of its batch (from a host-rotated x^T so
the banned diagonal band lands at fixed key-tile loop positions on every
core, keeping the SPMD graph uniform; masks are per-core 0/1 input data),
and full attention + out_proj + residual + LayerNorm for its 512 queries.

Math transformations (validated against the reference in numpy):
  - k-bias dropped (softmax shift invariance); v-bias folded into
    out_proj bias; q scaled by 1/sqrt(hd) on host.
  - all matmul operands are bf16 (validated 9.7e-4 rel err with exact
    exp); accumulation stays fp32 in PSUM.
  - exp via the Schraudolph bit trick in bf16:
        e = bitcast_bf16(int16(round(s * 128*log2(e) + (16256 - c*128))))
    applied to EVERY tile (mixing exact exp and Schraudolph tiles breaks
    softmax's common-mode error cancellation: all-Schraudolph = 2.2e-3
    final rel err, half-and-half = 1.3e-2). Each [128,1024] tile is
    split between the ACT engine (Copy activation with scale+bias ->
    int16; Copy is in every ACT table so NO table loads) and the DVE
    (tensor_scalar mult+add -> int16), halving the per-tile latency and
    splitting the elementwise bottleneck across two engines.
  - banned positions are zeroed AFTER exp via 0/1 bf16 mask multiply on
    the DVE (band strips only).
  - softmax denominator via a ones-column appended to v (row 64 of the
    65-row ctx accumulator); 1/s via a raw ACT Reciprocal (1e-5 rel err
    measured on HW; bass blocks it but it is fine at this tolerance),
    broadcast over 64 partitions with a PE outer product.
    NOTE: vector.reciprocal_approx_fast returns garbage on HW when its
    input AP is in PSUM (fine in CoreSim, fine from SBUF) - avoid.
  - LayerNorm rstd via raw ACT Rsqrt(var+eps) - one table load total.
"""

import ml_dtypes
import numpy as np

import concourse.bass as bass
import concourse.tile as tile
import concourse.mybir as mybir
from concourse import bacc
from concourse.bass_utils import run_bass_kernel_spmd

F32 = mybir.dt.float32
BF16 = mybir.dt.bfloat16
I16 = mybir.dt.int16
AF = mybir.ActivationFunctionType
OP = mybir.AluOpType

B, L, D = 2, 2048, 512
H, HD = 8, 64
SH = L // 4            # 512-query shard per core
NJ = 16                # key tiles of 128 per sequence
MASK_SLOTS = [0, 1, 2, 3, 4, 15]   # key-tile loop positions that can carry the band
LN_EPS = 1e-5

# Schraudolph bf16 exp: e = bitcast_bf16(int16(s*SCHR_SCALE + SCHR_BIAS))
SCHR_SCALE = 128.0 * 1.4426950408889634
SCHR_BIAS = 16256.0 - 0.43 * 128.0

_COMPILED = None
LAST_RESULT = None
STRIPS = []


def _raw_act(nc, out_ap, in_ap, func, bias=0.0, scale=1.0):
    """Raw InstActivation emission for funcs the bass API blocks
    (Reciprocal / Rsqrt); measured 1e-5 max rel err on this HW."""
    ins = [nc.scalar.lower_ap(in_ap)]
    for arg in (bias, scale, 0.0):
        if isinstance(arg, float):
            ins.append(mybir.ImmediateValue(dtype=mybir.dt.float32, value=arg))
        else:
            ins.append(nc.scalar.lower_ap(arg))
    return nc.scalar.add_instruction(
        mybir.InstActivation(
            name=nc.get_next_instruction_name(),
            func=func,
            ins=ins,
            outs=[nc.scalar.lower_ap(out_ap)],
        )
    )


def _build(half, ln_trivial):
    # band strip column ranges per mask slot (compile-time, depends on half)
    global STRIPS
    STRIPS = []
    for j in MASK_SLOTS[:-1]:
        c0 = max(0, 128 * j - half + 1)
        c1 = min(SH, 128 * j + 128 + half)
        STRIPS.append((c0, max(c1, c0 + 1)))
    STRIPS.append((0, max(1, min(SH, half))))

    nc = bacc.Bacc("TRN2", target_bir_lowering=False, debug=False, num_devices=8)

    xT = nc.dram_tensor("xT", [D, L], BF16, kind="ExternalInput")          # rotated x^T
    x_nat = nc.dram_tensor("x_nat", [SH, D], F32, kind="ExternalInput")    # query shard rows
    winT = nc.dram_tensor("winT", [D, 3 * D], BF16, kind="ExternalInput")  # in_proj_w.T, q cols pre-scaled
    woutT = nc.dram_tensor("woutT", [D, D], BF16, kind="ExternalInput")    # out_proj_w.T
    bq_d = nc.dram_tensor("bq", [128, 4], F32, kind="ExternalInput")       # scaled q bias, chunked
    gamma_d = nc.dram_tensor("gamma", [128, D], F32, kind="ExternalInput")  # broadcast ln gamma
    beta_d = nc.dram_tensor("beta", [128, D], F32, kind="ExternalInput")   # broadcast ln beta
    masks_d = nc.dram_tensor("masks", [len(MASK_SLOTS), 128, SH], BF16, kind="ExternalInput")
    out_d = nc.dram_tensor("out", [SH, D], BF16, kind="ExternalOutput")

    with tile.TileContext(nc) as tc:
        with (
            tc.tile_pool(name="persist", bufs=1) as pp,
            tc.tile_pool(name="work", bufs=2) as wp,
            tc.tile_pool(name="kvsb", bufs=1) as kvsb,
            tc.tile_pool(name="expp", bufs=6) as ep,
        ):
            # ---- prefetch: small q-slices first so PE starts early ----
            bq_sb = pp.tile([128, 4], F32, tag="bq")
            nc.sync.dma_start(out=bq_sb, in_=bq_d[:, :])
            win_sb = []
            xT_sb = []
            for d in range(4):
                t = pp.tile([128, L], BF16, tag=f"xT{d}")
                xT_sb.append(t)
                w = pp.tile([128, 3 * D], BF16, tag=f"win{d}")
                win_sb.append(w)
            wq_sb = [w[:, 0:D] for w in win_sb]
            wkv_sb = [w[:, D:3 * D] for w in win_sb]
            # one DMA per weight chunk; first xT segment feeds qT
            for d in range(4):
                nc.sync.dma_start(out=xT_sb[d][:, 0:SH], in_=xT[128 * d:128 * d + 128, 0:SH])
            for d in range(4):
                nc.sync.dma_start(out=win_sb[d], in_=winT[128 * d:128 * d + 128, :])
            # xT columns the first kt/v emissions need, then the tail
            for d in range(4):
                nc.sync.dma_start(out=xT_sb[d][:, SH:2 * SH], in_=xT[128 * d:128 * d + 128, SH:2 * SH])
            for d in range(4):
                nc.sync.dma_start(out=xT_sb[d][:, 2 * SH:L], in_=xT[128 * d:128 * d + 128, 2 * SH:L])
            # masks (needed from j=0, ~13us in): single coalesced DMA
            mall = pp.tile([128, len(MASK_SLOTS), SH], BF16, tag="mall")
            nc.sync.dma_start(out=mall, in_=masks_d[:, :, :].rearrange("i p q -> p i q"))
            mask_sb = []
            for i in range(len(MASK_SLOTS)):
                c0, c1 = STRIPS[i]
                mask_sb.append(mall[:, i, c0:c1])
            # late loads (needed only at the tail): coalesced
            wout_all = pp.tile([128, 4, D], BF16, tag="woutall")
            nc.sync.dma_start(out=wout_all, in_=woutT.rearrange("(c p) d -> p c d", p=128))
            woutT_sb = [wout_all[:, p, :] for p in range(4)]
            xn_all = pp.tile([128, 4, D], F32, tag="xnall")
            nc.sync.dma_start(out=xn_all, in_=x_nat.rearrange("(c p) d -> p c d", p=128))
            x_nat_sb = [xn_all[:, qt, :] for qt in range(4)]
            if not ln_trivial:
                gamma_sb = pp.tile([128, D], F32, tag="gamma")
                nc.sync.dma_start(out=gamma_sb, in_=gamma_d[:, :])
                beta_sb = pp.tile([128, D], F32, tag="beta")
                nc.sync.dma_start(out=beta_sb, in_=beta_d[:, :])

            # ---- constants ----
            wup = pp.tile([128, 128], BF16, tag="wup")
            nc.vector.memset(wup, 0.001)
            wupr = pp.tile([128, D], BF16, tag="wupr")
            nc.vector.memset(wupr, 0.001)
            ones1 = pp.tile([1, HD], BF16, tag="ones1")
            nc.vector.memset(ones1, 1.0)
            eps_t = pp.tile([128, 1], F32, tag="eps")
            nc.vector.memset(eps_t, LN_EPS)

            onescol = pp.tile([128, 1], BF16, tag="onescol")
            nc.vector.memset(onescol, 1.0)

            # v: one big tile [128, NJ, H*(HD+1)]; ones columns preset once
            v_sb = kvsb.tile([128, NJ, H * (HD + 1)], BF16, tag="v", name="v")
            v4 = v_sb.rearrange("p t (h c) -> p t h c", c=HD + 1)
            for l2 in range(NJ):
                nc.vector.tensor_copy(
                    v4[:, l2, :, HD:HD + 1],
                    onescol.rearrange("p (a c) -> p a c", a=1).broadcast_to((128, H, 1)),
                )

            kt_sb = [kvsb.tile([128, L], BF16, tag=f"kt{c2}", name=f"kt{c2}") for c2 in range(4)]
            qT_sb = [pp.tile([128, SH], BF16, tag=f"qT{c}", name=f"qT{c}") for c in range(4)]
            ctxTs_sb = [pp.tile([128, SH], BF16, tag=f"ctxTs{p}", name=f"ctxTs{p}") for p in range(4)]

            # PE warm-up: ~3us of matmuls so the HAM clock is at 8/8 when
            # real work starts (DMAs land meanwhile)
            with tc.tile_pool(name="wups", bufs=1, space="PSUM") as wps:
                wu_ps = wps.tile([128, D], F32, tag="wu")
                for i in range(8):
                    nc.tensor.matmul(
                        wu_ps, wup, wupr,
                        start=(i == 0), stop=(i == 7),
                    )

            # ---- q^T: [D, SH] as 4 chunks of [128, SH] ----
            with tc.tile_pool(name="qps", bufs=2, space="PSUM") as qps:
                for c in range(4):
                    ps = qps.tile([128, SH], F32, tag="q")
                    for d in range(4):
                        nc.tensor.matmul(
                            ps,
                            wq_sb[d][:, 128 * c:128 * c + 128],
                            xT_sb[d][:, 0:SH],
                            start=(d == 0), stop=(d == 3),
                        )
                    nc.vector.tensor_scalar_add(qT_sb[c], ps, bq_sb[:, c:c + 1])

            # ---- attention: two head-group phases share all PSUM pools ----
            with (
                tc.tile_pool(name="ctxps", bufs=1, space="PSUM") as cxp,
                tc.tile_pool(name="scps", bufs=2, space="PSUM") as scp,
            ):
                def emit_kt(c2, seg2):
                    """k^T chunk c2, key columns [1024*seg2, 1024*seg2+1024)."""
                    ps = scp.tile([128, 2 * SH], F32, tag="sc", name=f"ktps{c2}_{seg2}")
                    for hf in range(2):
                        for d in range(4):
                            nc.tensor.matmul(
                                ps[:, SH * hf:SH * hf + SH],
                                wkv_sb[d][:, 128 * c2:128 * c2 + 128],
                                xT_sb[d][:, 1024 * seg2 + SH * hf:1024 * seg2 + SH * hf + SH],
                                start=(d == 0), stop=(d == 3),
                            )
                    nc.scalar.copy(
                        kt_sb[c2][:, 1024 * seg2:1024 * seg2 + SH],
                        ps[:, 0:SH],
                    )
                    nc.vector.tensor_copy(
                        kt_sb[c2][:, 1024 * seg2 + SH:1024 * seg2 + 2 * SH],
                        ps[:, SH:2 * SH],
                    )

                def emit_v(pr):
                    """v for key tiles 2*pr and 2*pr+1."""
                    ps = scp.tile([128, 2 * SH], F32, tag="sc", name=f"vps{pr}")
                    for idx in range(2):
                        l2 = 2 * pr + idx
                        for d in range(4):
                            nc.tensor.matmul(
                                ps[:, SH * idx:SH * idx + SH],
                                xT_sb[d][:, 128 * l2:128 * l2 + 128],
                                wkv_sb[d][:, D:2 * D],
                                start=(d == 0), stop=(d == 3),
                            )
                    for idx in range(2):
                        nc.vector.tensor_copy(
                            v4[:, 2 * pr + idx, :, 0:HD],
                            ps[:, SH * idx:SH * idx + SH].rearrange("p (h c) -> p h c", c=HD),
                        )

                # deadline-ordered prep: v pair pr needed at j=2*pr; kt
                # group-0 seg2 needed at j=8*seg2; group-1 anytime before g1
                prep_queue = [
                    ("v", 1), ("v", 2), ("v", 3), ("v", 4), ("v", 5),
                    ("kt", (0, 1)), ("kt", (1, 1)),
                    ("v", 6), ("v", 7),
                    ("kt", (2, 0)), ("kt", (3, 0)), ("kt", (2, 1)), ("kt", (3, 1)),
                ]

                ctx_ps = [cxp.tile([HD + 1, SH], F32, tag=f"ctx{t}", name=f"ctx{t}") for t in range(4)]

                def emit_sc_exp(g, j, p2):
                    sc = scp.tile([128, 2 * SH], F32, tag="sc")
                    for t in range(2):
                        nc.tensor.matmul(
                            sc[:, SH * t:SH * t + SH],
                            kt_sb[2 * g + p2][64 * t:64 * t + 64, 128 * j:128 * j + 128],
                            qT_sb[2 * g + p2][64 * t:64 * t + 64, :],
                            start=True, stop=True,
                        )
                    e = ep.tile([128, 2 * SH], BF16, tag="exp")
                    ei = e.bitcast(I16)
                    # Schraudolph exp, halves on ACT + DVE concurrently
                    nc.scalar.activation(
                        ei[:, 0:SH], sc[:, 0:SH], AF.Copy,
                        bias=SCHR_BIAS, scale=SCHR_SCALE,
                    )
                    nc.vector.tensor_scalar(
                        out=ei[:, SH:2 * SH], in0=sc[:, SH:2 * SH],
                        scalar1=SCHR_SCALE, scalar2=SCHR_BIAS,
                        op0=OP.mult, op1=OP.add,
                    )
                    if j in MASK_SLOTS:
                        slot = MASK_SLOTS.index(j)
                        c0, c1 = STRIPS[slot]
                        w = c1 - c0
                        ev = e.rearrange("p (t q) -> p t q", t=2)[:, :, c0:c1]
                        mb = mask_sb[slot].rearrange(
                            "p (a q) -> p a q", a=1
                        ).broadcast_to((128, 2, w))
                        nc.gpsimd.tensor_tensor(out=ev, in0=ev, in1=mb, op=OP.mult)
                    return e

                def emit_ctx(g, j, p2, e):
                    for t in range(2):
                        ht = 2 * p2 + t
                        h = 4 * g + ht
                        nc.tensor.matmul(
                            ctx_ps[ht],
                            v_sb[:, j, (HD + 1) * h:(HD + 1) * h + HD + 1],
                            e[:, SH * t:SH * t + SH],
                            start=(j == 0), stop=(j == NJ - 1),
                        )

                def emit_divide(g):
                    # divide ctx by softmax sums, pack into ctxTs pair tiles
                    for p2 in range(2):
                        for t in range(2):
                            ht = 2 * p2 + t
                            recb = wp.tile([1, SH], BF16, tag="recb")
                            _raw_act(nc, recb[:, :], ctx_ps[ht][HD:HD + 1, :], AF.Reciprocal)
                            bc = scp.tile([128, 2 * SH], F32, tag="sc", name=f"bc{g}_{ht}")
                            nc.tensor.matmul(bc[0:HD, 0:SH], ones1, recb, start=True, stop=True)
                            bc_sb = wp.tile([HD, SH], F32, tag="bcsb")
                            nc.vector.tensor_copy(bc_sb, bc[0:HD, 0:SH])
                            nc.vector.tensor_tensor(
                                out=ctxTs_sb[2 * g + p2][64 * t:64 * t + 64, :],
                                in0=ctx_ps[ht][0:HD, :],
                                in1=bc_sb,
                                op=OP.mult,
                            )

                emit_kt(0, 0)
                emit_kt(1, 0)
                emit_v(0)
                PRE = 2  # g1 iterations whose scores are emitted before g0's
                         # divide, so the PE queue never head-of-line blocks
                         # on the reciprocal chain at the group transition
                pending = []
                for g in range(2):
                    if g == 1:
                        for j01 in range(PRE):
                            for p2 in range(2):
                                pending.append(emit_sc_exp(1, j01, p2))
                        emit_divide(0)
                    for j in range(NJ):
                        if g == 0 and prep_queue:
                            kind, a = prep_queue.pop(0)
                            if kind == "kt":
                                emit_kt(*a)
                            else:
                                emit_v(a)
                        for p2 in range(2):
                            if g == 1 and j < PRE:
                                e = pending.pop(0)
                            else:
                                e = emit_sc_exp(g, j, p2)
                            emit_ctx(g, j, p2, e)
                emit_divide(1)

            # ---- out_proj + residual + LayerNorm per query tile ----
            with tc.tile_pool(name="ops", bufs=2, space="PSUM") as ops:
                for qt in range(4):
                    po = ops.tile([128, D], F32, tag="po")
                    for p in range(4):
                        nc.tensor.matmul(
                            po,
                            ctxTs_sb[p][:, 128 * qt:128 * qt + 128],
                            woutT_sb[p],
                            start=(p == 0), stop=(p == 3),
                        )
                    # y = po + x_nat with row sums for free (mean);
                    # variance via ACT Square with accum; normalize on ACT.
                    y = wp.tile([128, D], F32, tag="y")
                    sums = wp.tile([128, 1], F32, tag="sums")
                    nc.vector.scalar_tensor_tensor(
                        out=y, in0=po, scalar=1.0, in1=x_nat_sb[qt],
                        op0=OP.mult, op1=OP.add, accum_out=sums,
                    )
                    sq = wp.tile([128, D], BF16, tag="sq")
                    sumsq = wp.tile([128, 1], F32, tag="sumsq")
                    nc.scalar.activation(sq, y, AF.Square, accum_out=sumsq)
                    negb = wp.tile([128, 1], F32, tag="negb")
                    nc.vector.scalar_tensor_tensor(
                        out=negb, in0=sums, scalar=-1.0 / (D * D), in1=sums,
                        op0=OP.mult, op1=OP.mult,
                    )
                    nc.vector.tensor_scalar_add(negb, negb, LN_EPS)
                    rstd = wp.tile([128, 1], F32, tag="rstd")
                    _raw_act(nc, rstd[:, :], sumsq[:, :], AF.Rsqrt,
                             bias=negb[:, :], scale=1.0 / D)
                    negmr = wp.tile([128, 1], F32, tag="negmr")
                    nc.vector.tensor_scalar(
                        out=negmr, in0=sums, scalar1=rstd[:, :], scalar2=-1.0 / D,
                        op0=OP.mult, op1=OP.mult,
                    )
                    t1 = wp.tile([128, D], BF16, tag="t1")
                    nc.scalar.activation(t1, y, AF.Identity,
                                         bias=negmr[:, :], scale=rstd[:, :])
                    if not ln_trivial:
                        nc.vector.tensor_tensor(out=t1, in0=t1, in1=gamma_sb, op=OP.mult)
                        nc.vector.tensor_tensor(out=t1, in0=t1, in1=beta_sb, op=OP.add)
                    nc.sync.dma_start(out=out_d[128 * qt:128 * qt + 128, :], in_=t1)

    nc.compile()
    return nc


def _host_prep(x, in_proj_w, in_proj_b, out_proj_w, out_proj_b, ln_gamma, ln_beta, window_size):
    x = np.ascontiguousarray(np.asarray(x, dtype=np.float32))
    in_proj_w = np.asarray(in_proj_w, dtype=np.float32)
    in_proj_b = np.asarray(in_proj_b, dtype=np.float32)
    out_proj_w = np.asarray(out_proj_w, dtype=np.float32)
    out_proj_b = np.asarray(out_proj_b, dtype=np.float32)
    ln_gamma = np.asarray(ln_gamma, dtype=np.float32)
    ln_beta = np.asarray(ln_beta, dtype=np.float32)
    w = int(np.asarray(window_size))
    half = w // 2
    assert half <= 128, "mask slots only cover |k-q| <= 128"

    bf = ml_dtypes.bfloat16
    scale = np.float32(1.0 / np.sqrt(HD))
    W = in_proj_w.copy()
    W[0:D] *= scale
    winT = np.ascontiguousarray(W.T.astype(bf))                # [D, 3D]
    woutT = np.ascontiguousarray(out_proj_w.T.astype(bf))      # [D, D]
    bq = np.ascontiguousarray((in_proj_b[0:D] * scale).reshape(4, 128).T)  # [128, 4]
    bout = (out_proj_b + out_proj_w @ in_proj_b[2 * D:3 * D]).reshape(1, D)
    gamma_b = np.ascontiguousarray(np.broadcast_to(ln_gamma, (128, D)))
    beta_b = np.ascontiguousarray(np.broadcast_to(ln_beta, (128, D)))

    in_maps = []
    for c in range(8):
        b, s = divmod(c, 4)
        rot = (SH * s + np.arange(L)) % L
        xT_rot = np.ascontiguousarray(x[b][rot].T.astype(bf))   # [D, L]
        x_nat = np.ascontiguousarray(x[b][SH * s:SH * s + SH] + bout[None, 0, :])  # [SH, D]
        masks = np.empty((len(MASK_SLOTS), 128, SH), np.float32)
        q_true = SH * s + np.arange(SH)[None, :]
        for i, j in enumerate(MASK_SLOTS):
            k_true = (SH * s + 128 * j + np.arange(128)[:, None]) % L
            dd = k_true - q_true
            banned = (dd >= -half) & (dd < half)
            masks[i] = 1.0 - banned.astype(np.float32)
        in_maps.append({
            "xT": xT_rot, "x_nat": x_nat, "winT": winT, "woutT": woutT,
            "bq": bq, "gamma": gamma_b, "beta": beta_b,
            "masks": masks.astype(bf),
        })
    return in_maps


def kernel(x, in_proj_w, in_proj_b, out_proj_w, out_proj_b, ln_gamma, ln_beta, window_size):
    global _COMPILED, LAST_RESULT
    half = int(np.asarray(window_size)) // 2
    ln_trivial = bool(np.all(np.asarray(ln_gamma) == 1.0) and np.all(np.asarray(ln_beta) == 0.0))
    key = (half, ln_trivial)
    if _COMPILED is None or _COMPILED[0] != key:
        _COMPILED = (key, _build(half, ln_trivial))
    in_maps = _host_prep(x, in_proj_w, in_proj_b, out_proj_w, out_proj_b,
                         ln_gamma, ln_beta, window_size)
    res = run_bass_kernel_spmd(_COMPILED[1], in_maps, core_ids=list(range(8)))
    LAST_RESULT = res
    out = np.empty((B, L, D), np.float32)
    for c in range(8):
        b, s = divmod(c, 4)
        out[b, SH * s:SH * s + SH] = np.asarray(res.results[c]["out"], dtype=np.float32)
    return out


# revision 22
# speedup vs baseline: 1.0591x; 1.0591x over previous
"""Trainium2 Bass kernel for LocalSparseAttention (anti-local windowed attention).

Reference computation (B=2, L=2048, D=512, H=8, hd=64):
    qkv = x @ in_proj_w.T + in_proj_b ; q,k,v = split(qkv)
    q *= 1/sqrt(hd)
    scores = q @ k.T  per head, with positions j in [i-w/2, i+w/2) BANNED (-inf)
    attn = softmax(scores); ctx = attn @ v
    out = LayerNorm(x + ctx @ out_proj_w.T + out_proj_b) * gamma + beta

Sharding: 8 cores = 2 batches x 4 query-shards of 512 rows. Each core
computes k/v for all 2048 keys of its batch (from a host-rotated x^T so
the banned diagonal band lands at fixed key-tile loop positions on every
core, keeping the SPMD graph uniform; masks are per-core 0/1 input data),
and full attention + out_proj + residual + LayerNorm for its 512 queries.

Math transformations (validated against the reference in numpy):
  - k-bias dropped (softmax shift invariance); v-bias folded into
    out_proj bias; q scaled by 1/sqrt(hd) on host.
  - all matmul operands are bf16 (validated 9.7e-4 rel err with exact
    exp); accumulation stays fp32 in PSUM.
  - exp via the Schraudolph bit trick in bf16:
        e = bitcast_bf16(int16(round(s * 128*log2(e) + (16256 - c*128))))
    applied to EVERY tile (mixing exact exp and Schraudolph tiles breaks
    softmax's common-mode error cancellation: all-Schraudolph = 2.2e-3
    final rel err, half-and-half = 1.3e-2). Each [128,1024] tile is
    split between the ACT engine (Copy activation with scale+bias ->
    int16; Copy is in every ACT table so NO table loads) and the DVE
    (tensor_scalar mult+add -> int16), halving the per-tile latency and
    splitting the elementwise bottleneck across two engines.
  - banned positions are zeroed AFTER exp via 0/1 bf16 mask multiply on
    the DVE (band strips only).
  - softmax denominator via a ones-column appended to v (row 64 of the
    65-row ctx accumulator); 1/s via a raw ACT Reciprocal (1e-5 rel err
    measured on HW; bass blocks it but it is fine at this tolerance),
    broadcast over 64 partitions with a PE outer product.
    NOTE: vector.reciprocal_approx_fast returns garbage on HW when its
    input AP is in PSUM (fine in CoreSim, fine from SBUF) - avoid.
  - LayerNorm rstd via raw ACT Rsqrt(var+eps) - one table load total.
"""

import ml_dtypes
import numpy as np

import concourse.bass as bass
import concourse.tile as tile
import concourse.mybir as mybir
from concourse import bacc
from concourse.bass_utils import run_bass_kernel_spmd

F32 = mybir.dt.float32
BF16 = mybir.dt.bfloat16
I16 = mybir.dt.int16
AF = mybir.ActivationFunctionType
OP = mybir.AluOpType

B, L, D = 2, 2048, 512
H, HD = 8, 64
SH = L // 4            # 512-query shard per core
NJ = 16                # key tiles of 128 per sequence
MASK_SLOTS = [0, 1, 2, 3, 4, 15]   # key-tile loop positions that can carry the band
LN_EPS = 1e-5

# Schraudolph bf16 exp: e = bitcast_bf16(int16(s*SCHR_SCALE + SCHR_BIAS))
SCHR_SCALE = 128.0 * 1.4426950408889634
SCHR_BIAS = 16256.0 - 0.43 * 128.0

_COMPILED = None
LAST_RESULT = None
STRIPS = []


def _raw_act(nc, out_ap, in_ap, func, bias=0.0, scale=1.0):
    """Raw InstActivation emission for funcs the bass API blocks
    (Reciprocal / Rsqrt); measured 1e-5 max rel err on this HW."""
    ins = [nc.scalar.lower_ap(in_ap)]
    for arg in (bias, scale, 0.0):
        if isinstance(arg, float):
            ins.append(mybir.ImmediateValue(dtype=mybir.dt.float32, value=arg))
        else:
            ins.append(nc.scalar.lower_ap(arg))
    return nc.scalar.add_instruction(
        mybir.InstActivation(
            name=nc.get_next_instruction_name(),
            func=func,
            ins=ins,
            outs=[nc.scalar.lower_ap(out_ap)],
        )
    )


def _build(half, ln_trivial):
    # band strip column ranges per mask slot (compile-time, depends on half)
    global STRIPS
    STRIPS = []
    for j in MASK_SLOTS[:-1]:
        c0 = max(0, 128 * j - half + 1)
        c1 = min(SH, 128 * j + 128 + half)
        STRIPS.append((c0, max(c1, c0 + 1)))
    STRIPS.append((0, max(1, min(SH, half))))

    nc = bacc.Bacc("TRN2", target_bir_lowering=False, debug=False, num_devices=8)

    xT = nc.dram_tensor("xT", [D, L], BF16, kind="ExternalInput")          # rotated x^T
    x_nat = nc.dram_tensor("x_nat", [SH, D], F32, kind="ExternalInput")    # query shard rows
    winT = nc.dram_tensor("winT", [D, 3 * D], BF16, kind="ExternalInput")  # in_proj_w.T, q cols pre-scaled
    woutT = nc.dram_tensor("woutT", [D, D], BF16, kind="ExternalInput")    # out_proj_w.T
    bq_d = nc.dram_tensor("bq", [128, 4], F32, kind="ExternalInput")       # scaled q bias, chunked
    gamma_d = nc.dram_tensor("gamma", [128, D], F32, kind="ExternalInput")  # broadcast ln gamma
    beta_d = nc.dram_tensor("beta", [128, D], F32, kind="ExternalInput")   # broadcast ln beta
    masks_d = nc.dram_tensor("masks", [len(MASK_SLOTS), 128, SH], BF16, kind="ExternalInput")
    out_d = nc.dram_tensor("out", [SH, D], BF16, kind="ExternalOutput")

    with tile.TileContext(nc) as tc:
        with (
            tc.tile_pool(name="persist", bufs=1) as pp,
            tc.tile_pool(name="work", bufs=2) as wp,
            tc.tile_pool(name="kvsb", bufs=1) as kvsb,
            tc.tile_pool(name="expp", bufs=6) as ep,
        ):
            # ---- prefetch: small q-slices first so PE starts early ----
            bq_sb = pp.tile([128, 4], F32, tag="bq")
            nc.sync.dma_start(out=bq_sb, in_=bq_d[:, :])
            win_sb = []
            xT_sb = []
            for d in range(4):
                t = pp.tile([128, L], BF16, tag=f"xT{d}")
                xT_sb.append(t)
                w = pp.tile([128, 3 * D], BF16, tag=f"win{d}")
                win_sb.append(w)
            wq_sb = [w[:, 0:D] for w in win_sb]
            wkv_sb = [w[:, D:3 * D] for w in win_sb]
            # one DMA per weight chunk; first xT segment feeds qT
            for d in range(4):
                nc.sync.dma_start(out=xT_sb[d][:, 0:SH], in_=xT[128 * d:128 * d + 128, 0:SH])
            for d in range(4):
                nc.sync.dma_start(out=win_sb[d], in_=winT[128 * d:128 * d + 128, :])
            # xT columns the first kt/v emissions need, then the tail
            for d in range(4):
                nc.sync.dma_start(out=xT_sb[d][:, SH:2 * SH], in_=xT[128 * d:128 * d + 128, SH:2 * SH])
            for d in range(4):
                nc.sync.dma_start(out=xT_sb[d][:, 2 * SH:L], in_=xT[128 * d:128 * d + 128, 2 * SH:L])
            # masks (needed from j=0, ~13us in): single coalesced DMA
            mall = pp.tile([128, len(MASK_SLOTS), SH], BF16, tag="mall")
            nc.sync.dma_start(out=mall, in_=masks_d[:, :, :].rearrange("i p q -> p i q"))
            mask_sb = []
            for i in range(len(MASK_SLOTS)):
                c0, c1 = STRIPS[i]
                mask_sb.append(mall[:, i, c0:c1])
            # late loads (needed only at the tail): coalesced
            wout_all = pp.tile([128, 4, D], BF16, tag="woutall")
            nc.sync.dma_start(out=wout_all, in_=woutT.rearrange("(c p) d -> p c d", p=128))
            woutT_sb = [wout_all[:, p, :] for p in range(4)]
            xn_all = pp.tile([128, 4, D], F32, tag="xnall")
            nc.sync.dma_start(out=xn_all, in_=x_nat.rearrange("(c p) d -> p c d", p=128))
            x_nat_sb = [xn_all[:, qt, :] for qt in range(4)]
            if not ln_trivial:
                gamma_sb = pp.tile([128, D], F32, tag="gamma")
                nc.sync.dma_start(out=gamma_sb, in_=gamma_d[:, :])
                beta_sb = pp.tile([128, D], F32, tag="beta")
                nc.sync.dma_start(out=beta_sb, in_=beta_d[:, :])

            # ---- constants ----
            wup = pp.tile([128, 128], BF16, tag="wup")
            nc.vector.memset(wup, 0.001)
            wupr = pp.tile([128, D], BF16, tag="wupr")
            nc.vector.memset(wupr, 0.001)
            ones1 = pp.tile([1, HD], BF16, tag="ones1")
            nc.vector.memset(ones1, 1.0)
            eps_t = pp.tile([128, 1], F32, tag="eps")
            nc.vector.memset(eps_t, LN_EPS)

            onescol = pp.tile([128, 1], BF16, tag="onescol")
            nc.vector.memset(onescol, 1.0)

            # v: one big tile [128, NJ, H*(HD+1)]; ones columns preset once
            v_sb = kvsb.tile([128, NJ, H * (HD + 1)], BF16, tag="v", name="v")
            v4 = v_sb.rearrange("p t (h c) -> p t h c", c=HD + 1)
            for l2 in range(NJ):
                nc.vector.tensor_copy(
                    v4[:, l2, :, HD:HD + 1],
                    onescol.rearrange("p (a c) -> p a c", a=1).broadcast_to((128, H, 1)),
                )

            kt_sb = [kvsb.tile([128, L], BF16, tag=f"kt{c2}", name=f"kt{c2}") for c2 in range(4)]
            qT_sb = [pp.tile([128, SH], BF16, tag=f"qT{c}", name=f"qT{c}") for c in range(4)]
            ctxTs_sb = [pp.tile([128, SH], BF16, tag=f"ctxTs{p}", name=f"ctxTs{p}") for p in range(4)]

            # PE warm-up: ~3us of matmuls so the HAM clock is at 8/8 when
            # real work starts (DMAs land meanwhile)
            with tc.tile_pool(name="wups", bufs=1, space="PSUM") as wps:
                wu_ps = wps.tile([128, D], F32, tag="wu")
                for i in range(8):
                    nc.tensor.matmul(
                        wu_ps, wup, wupr,
                        start=(i == 0), stop=(i == 7),
                    )

            # ---- q^T: [D, SH] as 4 chunks of [128, SH] ----
            with tc.tile_pool(name="qps", bufs=2, space="PSUM") as qps:
                for c in range(4):
                    ps = qps.tile([128, SH], F32, tag="q")
                    for d in range(4):
                        nc.tensor.matmul(
                            ps,
                            wq_sb[d][:, 128 * c:128 * c + 128],
                            xT_sb[d][:, 0:SH],
                            start=(d == 0), stop=(d == 3),
                        )
                    nc.vector.tensor_scalar_add(qT_sb[c], ps, bq_sb[:, c:c + 1])

            # ---- attention: two head-group phases share all PSUM pools ----
            with (
                tc.tile_pool(name="ctxps", bufs=1, space="PSUM") as cxp,
                tc.tile_pool(name="scps", bufs=2, space="PSUM") as scp,
            ):
                def emit_kt(c2, seg2):
                    """k^T chunk c2, key columns [1024*seg2, 1024*seg2+1024)."""
                    ps = scp.tile([128, 2 * SH], F32, tag="sc", name=f"ktps{c2}_{seg2}")
                    for hf in range(2):
                        for d in range(4):
                            nc.tensor.matmul(
                                ps[:, SH * hf:SH * hf + SH],
                                wkv_sb[d][:, 128 * c2:128 * c2 + 128],
                                xT_sb[d][:, 1024 * seg2 + SH * hf:1024 * seg2 + SH * hf + SH],
                                start=(d == 0), stop=(d == 3),
                            )
                    nc.scalar.copy(
                        kt_sb[c2][:, 1024 * seg2:1024 * seg2 + SH],
                        ps[:, 0:SH],
                    )
                    nc.vector.tensor_copy(
                        kt_sb[c2][:, 1024 * seg2 + SH:1024 * seg2 + 2 * SH],
                        ps[:, SH:2 * SH],
                    )

                def emit_v(pr):
                    """v for key tiles 2*pr and 2*pr+1."""
                    ps = scp.tile([128, 2 * SH], F32, tag="sc", name=f"vps{pr}")
                    for idx in range(2):
                        l2 = 2 * pr + idx
                        for d in range(4):
                            nc.tensor.matmul(
                                ps[:, SH * idx:SH * idx + SH],
                                xT_sb[d][:, 128 * l2:128 * l2 + 128],
                                wkv_sb[d][:, D:2 * D],
                                start=(d == 0), stop=(d == 3),
                            )
                    for idx in range(2):
                        nc.vector.tensor_copy(
                            v4[:, 2 * pr + idx, :, 0:HD],
                            ps[:, SH * idx:SH * idx + SH].rearrange("p (h c) -> p h c", c=HD),
                        )

                # deadline-ordered prep: v pair pr needed at j=2*pr; kt
                # group-0 seg2 needed at j=8*seg2; group-1 anytime before g1
                prep_queue = [
                    ("v", 1), ("v", 2), ("v", 3), ("v", 4), ("v", 5),
                    ("kt", (0, 1)), ("kt", (1, 1)),
                    ("v", 6), ("v", 7),
                    ("kt", (2, 0)), ("kt", (3, 0)), ("kt", (2, 1)), ("kt", (3, 1)),
                ]

                ctx_ps = [cxp.tile([HD + 1, SH], F32, tag=f"ctx{t}", name=f"ctx{t}") for t in range(4)]

                def emit_sc_exp(g, j, p2):
                    sc = scp.tile([128, 2 * SH], F32, tag="sc")
                    for t in range(2):
                        nc.tensor.matmul(
                            sc[:, SH * t:SH * t + SH],
                            kt_sb[2 * g + p2][64 * t:64 * t + 64, 128 * j:128 * j + 128],
                            qT_sb[2 * g + p2][64 * t:64 * t + 64, :],
                            start=True, stop=True,
                        )
                    e = ep.tile([128, 2 * SH], BF16, tag="exp")
                    ei = e.bitcast(I16)
                    # Schraudolph exp, halves on ACT + DVE concurrently
                    nc.scalar.activation(
                        ei[:, 0:SH], sc[:, 0:SH], AF.Copy,
                        bias=SCHR_BIAS, scale=SCHR_SCALE,
                    )
                    nc.vector.tensor_scalar(
                        out=ei[:, SH:2 * SH], in0=sc[:, SH:2 * SH],
                        scalar1=SCHR_SCALE, scalar2=SCHR_BIAS,
                        op0=OP.mult, op1=OP.add,
                    )
                    if j in MASK_SLOTS:
                        slot = MASK_SLOTS.index(j)
                        c0, c1 = STRIPS[slot]
                        w = c1 - c0
                        ev = e.rearrange("p (t q) -> p t q", t=2)[:, :, c0:c1]
                        mb = mask_sb[slot].rearrange(
                            "p (a q) -> p a q", a=1
                        ).broadcast_to((128, 2, w))
                        nc.vector.tensor_tensor(out=ev, in0=ev, in1=mb, op=OP.mult)
                    return e

                def emit_ctx(g, j, p2, e):
                    for t in range(2):
                        ht = 2 * p2 + t
                        h = 4 * g + ht
                        nc.tensor.matmul(
                            ctx_ps[ht],
                            v_sb[:, j, (HD + 1) * h:(HD + 1) * h + HD + 1],
                            e[:, SH * t:SH * t + SH],
                            start=(j == 0), stop=(j == NJ - 1),
                        )

                def emit_divide(g):
                    # divide ctx by softmax sums, pack into ctxTs pair tiles
                    for p2 in range(2):
                        for t in range(2):
                            ht = 2 * p2 + t
                            recb = wp.tile([1, SH], BF16, tag="recb")
                            _raw_act(nc, recb[:, :], ctx_ps[ht][HD:HD + 1, :], AF.Reciprocal)
                            bc = scp.tile([128, 2 * SH], F32, tag="sc", name=f"bc{g}_{ht}")
                            nc.tensor.matmul(bc[0:HD, 0:SH], ones1, recb, start=True, stop=True)
                            bc_sb = wp.tile([HD, SH], F32, tag="bcsb")
                            nc.vector.tensor_copy(bc_sb, bc[0:HD, 0:SH])
                            nc.vector.tensor_tensor(
                                out=ctxTs_sb[2 * g + p2][64 * t:64 * t + 64, :],
                                in0=ctx_ps[ht][0:HD, :],
                                in1=bc_sb,
                                op=OP.mult,
                            )

                emit_kt(0, 0)
                emit_kt(1, 0)
                emit_v(0)
                PRE = 2  # g1 iterations whose scores are emitted before g0's
                         # divide, so the PE queue never head-of-line blocks
                         # on the reciprocal chain at the group transition
                pending = []
                for g in range(2):
                    if g == 1:
                        for j01 in range(PRE):
                            for p2 in range(2):
                                pending.append(emit_sc_exp(1, j01, p2))
                        emit_divide(0)
                    for j in range(NJ):
                        if g == 0 and prep_queue:
                            kind, a = prep_queue.pop(0)
                            if kind == "kt":
                                emit_kt(*a)
                            else:
                                emit_v(a)
                        for p2 in range(2):
                            if g == 1 and j < PRE:
                                e = pending.pop(0)
                            else:
                                e = emit_sc_exp(g, j, p2)
                            emit_ctx(g, j, p2, e)
                emit_divide(1)

            # ---- out_proj + residual + LayerNorm per query tile ----
            with tc.tile_pool(name="ops", bufs=2, space="PSUM") as ops:
                for qt in range(4):
                    po = ops.tile([128, D], F32, tag="po")
                    for p in range(4):
                        nc.tensor.matmul(
                            po,
                            ctxTs_sb[p][:, 128 * qt:128 * qt + 128],
                            woutT_sb[p],
                            start=(p == 0), stop=(p == 3),
                        )
                    # y = po + x_nat with row sums for free (mean);
                    # variance via ACT Square with accum; normalize on ACT.
                    y = wp.tile([128, D], F32, tag="y")
                    sums = wp.tile([128, 1], F32, tag="sums")
                    nc.vector.scalar_tensor_tensor(
                        out=y, in0=po, scalar=1.0, in1=x_nat_sb[qt],
                        op0=OP.mult, op1=OP.add, accum_out=sums,
                    )
                    sq = wp.tile([128, D], BF16, tag="sq")
                    sumsq = wp.tile([128, 1], F32, tag="sumsq")
                    nc.scalar.activation(sq, y, AF.Square, accum_out=sumsq)
                    negb = wp.tile([128, 1], F32, tag="negb")
                    nc.vector.scalar_tensor_tensor(
                        out=negb, in0=sums, scalar=-1.0 / (D * D), in1=sums,
                        op0=OP.mult, op1=OP.mult,
                    )
                    nc.vector.tensor_scalar_add(negb, negb, LN_EPS)
                    rstd = wp.tile([128, 1], F32, tag="rstd")
                    _raw_act(nc, rstd[:, :], sumsq[:, :], AF.Rsqrt,
                             bias=negb[:, :], scale=1.0 / D)
                    negmr = wp.tile([128, 1], F32, tag="negmr")
                    nc.vector.tensor_scalar(
                        out=negmr, in0=sums, scalar1=rstd[:, :], scalar2=-1.0 / D,
                        op0=OP.mult, op1=OP.mult,
                    )
                    t1 = wp.tile([128, D], BF16, tag="t1")
                    nc.scalar.activation(t1, y, AF.Identity,
                                         bias=negmr[:, :], scale=rstd[:, :])
                    if not ln_trivial:
                        nc.vector.tensor_tensor(out=t1, in0=t1, in1=gamma_sb, op=OP.mult)
                        nc.vector.tensor_tensor(out=t1, in0=t1, in1=beta_sb, op=OP.add)
                    nc.sync.dma_start(out=out_d[128 * qt:128 * qt + 128, :], in_=t1)

    nc.compile()
    return nc


def _host_prep(x, in_proj_w, in_proj_b, out_proj_w, out_proj_b, ln_gamma, ln_beta, window_size):
    x = np.ascontiguousarray(np.asarray(x, dtype=np.float32))
    in_proj_w = np.asarray(in_proj_w, dtype=np.float32)
    in_proj_b = np.asarray(in_proj_b, dtype=np.float32)
    out_proj_w = np.asarray(out_proj_w, dtype=np.float32)
    out_proj_b = np.asarray(out_proj_b, dtype=np.float32)
    ln_gamma = np.asarray(ln_gamma, dtype=np.float32)
    ln_beta = np.asarray(ln_beta, dtype=np.float32)
    w = int(np.asarray(window_size))
    half = w // 2
    assert half <= 128, "mask slots only cover |k-q| <= 128"

    bf = ml_dtypes.bfloat16
    scale = np.float32(1.0 / np.sqrt(HD))
    W = in_proj_w.copy()
    W[0:D] *= scale
    winT = np.ascontiguousarray(W.T.astype(bf))                # [D, 3D]
    woutT = np.ascontiguousarray(out_proj_w.T.astype(bf))      # [D, D]
    bq = np.ascontiguousarray((in_proj_b[0:D] * scale).reshape(4, 128).T)  # [128, 4]
    bout = (out_proj_b + out_proj_w @ in_proj_b[2 * D:3 * D]).reshape(1, D)
    gamma_b = np.ascontiguousarray(np.broadcast_to(ln_gamma, (128, D)))
    beta_b = np.ascontiguousarray(np.broadcast_to(ln_beta, (128, D)))

    in_maps = []
    for c in range(8):
        b, s = divmod(c, 4)
        rot = (SH * s + np.arange(L)) % L
        xT_rot = np.ascontiguousarray(x[b][rot].T.astype(bf))   # [D, L]
        x_nat = np.ascontiguousarray(x[b][SH * s:SH * s + SH] + bout[None, 0, :])  # [SH, D]
        masks = np.empty((len(MASK_SLOTS), 128, SH), np.float32)
        q_true = SH * s + np.arange(SH)[None, :]
        for i, j in enumerate(MASK_SLOTS):
            k_true = (SH * s + 128 * j + np.arange(128)[:, None]) % L
            dd = k_true - q_true
            banned = (dd >= -half) & (dd < half)
            masks[i] = 1.0 - banned.astype(np.float32)
        in_maps.append({
            "xT": xT_rot, "x_nat": x_nat, "winT": winT, "woutT": woutT,
            "bq": bq, "gamma": gamma_b, "beta": beta_b,
            "masks": masks.astype(bf),
        })
    return in_maps


def kernel(x, in_proj_w, in_proj_b, out_proj_w, out_proj_b, ln_gamma, ln_beta, window_size):
    global _COMPILED, LAST_RESULT
    half = int(np.asarray(window_size)) // 2
    ln_trivial = bool(np.all(np.asarray(ln_gamma) == 1.0) and np.all(np.asarray(ln_beta) == 0.0))
    key = (half, ln_trivial)
    if _COMPILED is None or _COMPILED[0] != key:
        _COMPILED = (key, _build(half, ln_trivial))
    in_maps = _host_prep(x, in_proj_w, in_proj_b, out_proj_w, out_proj_b,
                         ln_gamma, ln_beta, window_size)
    res = run_bass_kernel_spmd(_COMPILED[1], in_maps, core_ids=list(range(8)))
    LAST_RESULT = res
    out = np.empty((B, L, D), np.float32)
    for c in range(8):
        b, s = divmod(c, 4)
        out[b, SH * s:SH * s + SH] = np.asarray(res.results[c]["out"], dtype=np.float32)
    return out
